# revision 1
# baseline (speedup 1.0000x reference)
"""Trainium2 Bass kernel for nn_Evolution_26697516712465 (deep-snake GNN).

Self-contained: takes FULL inputs, shards batch across 8 NeuronCores internally
(one image per core; each core runs the snake for the polys of its own image),
returns FULL output [128, 128, 2] fp32.
"""
import numpy as np
import ml_dtypes
from contextlib import ExitStack

import concourse.bass as bass
import concourse.bacc as bacc
import concourse.mybir as mybir
import concourse.tile as tile
from concourse.library_config import mlp as mlp_lib
from concourse.bass_utils import run_bass_kernel_spmd

N_CORES = 8
B, C_IN, H, W = 8, 66, 128, 128
NP, V = 128, 128
RO = 4.0
DIL = (1, 1, 1, 2, 2, 4, 4)
NRES = 7
HW = H * W          # 16384
PADW = W + 2        # 130
PIMG = PADW * PADW  # 16900
PADV = 160          # 16 + 128 + 16 circular pad

f32 = mybir.dt.float32
bf16 = mybir.dt.bfloat16
i16 = mybir.dt.int16
AF = mybir.ActivationFunctionType
ALU = mybir.AluOpType

BF = ml_dtypes.bfloat16


def _bcast(ap_obj, n):
    """Append a step-0 (broadcast) innermost free dim of size n to an AP."""
    return bass.AP(tensor=ap_obj.tensor, offset=ap_obj.offset,
                   ap=[*ap_obj.ap, [0, n]])


def build_nc(P):
    """Build the SPMD Bass program. P = max polys per image."""
    nc = bacc.Bacc("TRN2", target_bir_lowering=False, debug=False)
    NV = P * 128           # corner-gather idx count (multiple of 128)
    ICOLS = NV // 16
    PADQ = -(-P // 4) * 4  # snake poly slots (multiple of 4)
    NQB = PADQ // 4

    # ---------------- inputs ----------------
    d_stack0 = nc.declare_dram_parameter("stack0", [128, PIMG], bf16, isOutput=False)
    d_stack1 = nc.declare_dram_parameter("stack1", [70, PIMG], bf16, isOutput=False)
    d_w1p0 = nc.declare_dram_parameter("w1p0", [128, 3, 2, 128], bf16, isOutput=False)
    d_w1p1 = nc.declare_dram_parameter("w1p1", [70, 3, 2, 128], bf16, isOutput=False)
    d_w2t = nc.declare_dram_parameter("w2t", [128, 2, 64], bf16, isOutput=False)
    d_pb0 = nc.declare_dram_parameter("pb0", [128, 2], f32, isOutput=False)
    d_fusb = nc.declare_dram_parameter("fusb", [128, 2], f32, isOutput=False)
    d_idxc = nc.declare_dram_parameter("idxc", [128, 4, ICOLS], i16, isOutput=False)
    d_wcomp = nc.declare_dram_parameter("wcomp", [128, 4, P], f32, isOutput=False)
    d_b2s = nc.declare_dram_parameter("b2s", [128, P, 64], f32, isOutput=False)
    d_coords = nc.declare_dram_parameter("coords", [128, P, 2], bf16, isOutput=False)
    d_iidx = nc.declare_dram_parameter("iidx", [128, PADQ * PADV // 16], i16, isOutput=False)
    d_base = nc.declare_dram_parameter("base", [128, PADQ, 2], f32, isOutput=False)
    d_headw = nc.declare_dram_parameter("headw", [66, 9, 128], bf16, isOutput=False)
    d_headb = nc.declare_dram_parameter("headb", [128, 3], f32, isOutput=False)
    d_resw = nc.declare_dram_parameter("resw", [128, 63, 128], bf16, isOutput=False)
    d_resb = nc.declare_dram_parameter("resb", [128, 3, 7], f32, isOutput=False)
    d_fusw = nc.declare_dram_parameter("fusw", [128, 8, 256], bf16, isOutput=False)
    d_pw1 = nc.declare_dram_parameter("pw1", [128, 10, 256], bf16, isOutput=False)
    d_pb1 = nc.declare_dram_parameter("pb1", [128, 2], f32, isOutput=False)
    d_pw2 = nc.declare_dram_parameter("pw2", [128, 2, 64], bf16, isOutput=False)
    d_pb2 = nc.declare_dram_parameter("pb2", [64, 1], f32, isOutput=False)
    d_pw3 = nc.declare_dram_parameter("pw3", [64, 2], bf16, isOutput=False)
    d_out = nc.declare_dram_parameter("out", [128, PADQ, 2], f32, isOutput=True)

    feat_dram = nc.dram_tensor("feat_dram", [HW, 64], f32)
    cc_in = nc.dram_tensor("cc_in", [NV, 128], bf16)

    with tile.TileContext(nc, num_cores=N_CORES) as tc, ExitStack() as top:
        wpool = top.enter_context(tc.tile_pool(name="weights", bufs=1))
        w2t_t = wpool.tile([128, 2, 64], bf16)
        nc.sync.dma_start(out=w2t_t, in_=d_w2t[:, :, :])
        pb0_t = wpool.tile([128, 2], f32)
        nc.sync.dma_start(out=pb0_t, in_=d_pb0[:, :])
        fusb_t = wpool.tile([128, 2], f32)
        nc.sync.dma_start(out=fusb_t, in_=d_fusb[:, :])
        idxc_t = wpool.tile([128, 4, ICOLS], i16)
        nc.sync.dma_start(out=idxc_t, in_=d_idxc[:, :, :])
        wcomp_t = wpool.tile([128, 4, P], f32)
        nc.sync.dma_start(out=wcomp_t, in_=d_wcomp[:, :, :])
        b2s_t = wpool.tile([128, P, 64], f32)
        nc.sync.dma_start(out=b2s_t, in_=d_b2s[:, :, :])
        coords_t = wpool.tile([128, P, 2], bf16)
        nc.sync.dma_start(out=coords_t, in_=d_coords[:, :, :])
        iidx_t = wpool.tile([128, PADQ * PADV // 16], i16)
        nc.sync.dma_start(out=iidx_t, in_=d_iidx[:, :])
        base_t = wpool.tile([128, PADQ, 2], f32)
        nc.sync.dma_start(out=base_t, in_=d_base[:, :, :])
        headw_t = wpool.tile([66, 9, 128], bf16)
        headb_t = wpool.tile([128, 3], f32)
        resw_t = wpool.tile([128, 63, 128], bf16)
        resb_t = wpool.tile([128, 3, 7], f32)
        fusw_t = wpool.tile([128, 8, 256], bf16)
        pw1_t = wpool.tile([128, 10, 256], bf16)
        pb1_t = wpool.tile([128, 2], f32)
        pw2_t = wpool.tile([128, 2, 64], bf16)
        pb2_t = wpool.tile([64, 1], f32)
        pw3_t = wpool.tile([64, 2], bf16)

        nc.gpsimd.load_library(mlp_lib)

        # relu1 lives across conv1 + conv2
        with tc.tile_pool(name="relu1", bufs=1) as rpool:
            r1 = [rpool.tile([128, HW], bf16, tag=f"r1_{m}", name=f"r1_{m}")
                  for m in range(2)]

            # ------------ conv1: 3x3 66->256 (bf16, K packed 128+70) ------------
            with tc.tile_pool(name="stacks", bufs=1) as stpool, \
                 tc.tile_pool(name="psumA", bufs=3, space="PSUM") as ppA:
                st0 = stpool.tile([128, PIMG], bf16)
                HALF = 68 * PADW
                nc.sync.dma_start(out=st0[:, :HALF], in_=d_stack0[:, :HALF])
                nc.sync.dma_start(out=st0[:, HALF:], in_=d_stack0[:, HALF:])
                st1 = stpool.tile([70, PIMG], bf16)
                nc.sync.dma_start(out=st1[:, :HALF], in_=d_stack1[:, :HALF])
                nc.sync.dma_start(out=st1[:, HALF:], in_=d_stack1[:, HALF:])
                w1p0_t = stpool.tile([128, 3, 2, 128], bf16)
                nc.sync.dma_start(out=w1p0_t, in_=d_w1p0[:, :, :, :])
                w1p1_t = stpool.tile([70, 3, 2, 128], bf16)
                nc.sync.dma_start(out=w1p1_t, in_=d_w1p1[:, :, :, :])

                for t in range(32):          # hw tiles of 512 = 4 image rows
                    y0 = 4 * t
                    for m in range(2):       # out-channel half
                        ps = ppA.tile([128, 512], f32, tag="psA", name="psA")
                        i = 0
                        for (stk, wt) in ((st0, w1p0_t), (st1, w1p1_t)):
                            for kw in range(3):
                                rhs = bass.AP(tensor=stk.tensor,
                                              offset=stk.offset + y0 * PADW + kw,
                                              ap=[stk.ap[0], [PADW, 4], [1, 128]])
                                nc.tensor.matmul(ps, wt[:, kw, m, :], rhs,
                                                 start=(i == 0), stop=(i == 5))
                                i += 1
                        nc.scalar.activation(r1[m][:, t * 512:(t + 1) * 512], ps,
                                             AF.Relu, bias=pb0_t[:, m:m + 1])

            # ------------ conv2: 1x1 256->64, out [hw, 64] fp32 -> DRAM ------------
            with tc.tile_pool(name="psumB", bufs=2, space="PSUM") as ppB, \
                 tc.tile_pool(name="stage", bufs=3) as spool:
                for g in range(16):
                    ps2 = ppB.tile([128, 512], f32, tag="psB", name="psB")
                    for j in range(8):
                        hw0 = (g * 8 + j) * 128
                        for ch in range(2):
                            nc.tensor.matmul(ps2[:, j * 64:(j + 1) * 64],
                                             r1[ch][:, hw0:hw0 + 128],
                                             w2t_t[:, ch, :],
                                             start=(ch == 0), stop=(ch == 1))
                    stg = spool.tile([128, 512], f32, tag="stage", name="stg")
                    nc.vector.tensor_copy(stg, ps2)
                    dst = bass.AP(tensor=feat_dram, offset=g * 65536,
                                  ap=[[512, 128], [1, 512]])
                    nc.sync.dma_start(out=dst, in_=stg)

        # ------------ bilinear gather + weighted sum + vertex rows ------------
        with tc.tile_pool(name="gpool", bufs=1) as gpool:
            gts = []
            for c in range(4):
                gt = gpool.tile([128, P, 64], f32, tag=f"g{c}", name=f"g{c}")
                src = bass.AP(tensor=feat_dram, offset=0, ap=[[64, HW], [1, 64]])
                nc.gpsimd.dma_gather(gt, src, idxc_t[:, c, :], NV, NV, 64,
                                     single_packet=False)
                gts.append(gt)
            vert = gpool.tile([128, P, 64], f32, tag="vert", name="vert")
            tmp = gpool.tile([128, P, 64], f32, tag="tmp", name="tmp")
            for c in range(4):
                wb = _bcast(wcomp_t[:, c, :], 64)
                if c == 0:
                    nc.vector.tensor_tensor(vert, gts[c], wb, ALU.mult)
                else:
                    nc.vector.tensor_tensor(tmp, gts[c], wb, ALU.mult)
                    nc.vector.tensor_tensor(vert, vert, tmp, ALU.add)
            nc.vector.tensor_tensor(vert, vert, b2s_t, ALU.add)

            contrib = gpool.tile([128, P, 128], bf16, tag="contrib", name="contrib")
            nc.vector.memset(contrib, 0.0)
            nc.vector.tensor_copy(contrib[:, :, 0:64], vert)
            nc.vector.tensor_copy(contrib[:, :, 64:66], coords_t)
            # SBUF [v, q, ch] -> DRAM row q*128+v
            dst = bass.AP(tensor=cc_in, offset=0,
                          ap=[[128, 128], [128 * 128, P], [1, 128]])
            nc.sync.dma_start(out=dst, in_=contrib)

        # snake weights load late (off conv1's critical DMA path)
        nc.sync.dma_start(out=headw_t, in_=d_headw[:, :, :])
        nc.sync.dma_start(out=headb_t, in_=d_headb[:, :])
        nc.sync.dma_start(out=resw_t, in_=d_resw[:, :, :])
        nc.sync.dma_start(out=resb_t, in_=d_resb[:, :, :])
        nc.sync.dma_start(out=fusw_t, in_=d_fusw[:, :, :])
        nc.sync.dma_start(out=pw1_t, in_=d_pw1[:, :, :])
        nc.sync.dma_start(out=pb1_t, in_=d_pb1[:, :])
        nc.sync.dma_start(out=pw2_t, in_=d_pw2[:, :, :])
        nc.sync.dma_start(out=pb2_t, in_=d_pb2[:, :])
        nc.sync.dma_start(out=pw3_t, in_=d_pw3[:, :])

        # ---------------- snake ----------------
        with tc.tile_pool(name="snake", bufs=1) as sn, \
             tc.tile_pool(name="psumS", bufs=4, space="PSUM") as ppS, \
             tc.tile_pool(name="psumT", bufs=2, space="PSUM") as ppT:
            # init transpose-gather directly into circular-padded [ch, poly, 160]
            ipad_raw = sn.tile([128, 1, PADQ * PADV], bf16, tag="ipad", name="ipad")
            ccsrc = bass.AP(tensor=cc_in, offset=0, ap=[[128, NV], [1, 128]])
            nc.gpsimd.dma_gather(ipad_raw, ccsrc, iidx_t[:, :],
                                 PADQ * PADV, PADQ * PADV, 128, transpose=True,
                                 single_packet=False)
            ipad = ipad_raw[:, 0, :].rearrange("p (q k) -> p q k", k=PADV)

            spads = [sn.tile([128, PADQ, PADV], bf16, tag=f"spad{k}", name=f"spad{k}")
                     for k in range(8)]

            def circ_conv(dst_pad, src_pad, src_parts, lhsT_of_tap, bias_ap, gam_ap,
                          bet_ap, dilation, residual):
                for qb in range(NQB):
                    ps = ppS.tile([128, 512], f32, tag="psS", name="psS")
                    for t in range(9):
                        off = qb * 4 * PADV + 16 + (t - 4) * dilation
                        rhs = bass.AP(tensor=src_pad.tensor,
                                      offset=src_pad.offset + off,
                                      ap=[[src_pad.ap[0][0], src_parts],
                                          [PADV, 4], [1, 128]])
                        nc.tensor.matmul(ps, lhsT_of_tap(t), rhs,
                                         start=(t == 0), stop=(t == 8))
                    nc.scalar.activation(
                        dst_pad[:, qb * 4:(qb + 1) * 4, 16:144],
                        ps.rearrange("p (a b) -> p a b", a=4), AF.Relu, bias=bias_ap)
                ctr = dst_pad[:, :, 16:144]
                nc.vector.tensor_scalar(ctr, ctr, gam_ap, bet_ap,
                                        op0=ALU.mult, op1=ALU.add)
                if residual is not None:
                    nc.vector.tensor_tensor(ctr, ctr, residual[:, :, 16:144], ALU.add)
                nc.vector.tensor_copy(dst_pad[:, :, 0:16], dst_pad[:, :, 128:144])
                nc.vector.tensor_copy(dst_pad[:, :, 144:160], dst_pad[:, :, 16:32])

            circ_conv(spads[0], ipad[0:66], 66,
                      lambda t: headw_t[:, t, :],
                      headb_t[:, 0:1], headb_t[:, 1:2], headb_t[:, 2:3], 1, None)
            for i in range(NRES):
                circ_conv(spads[i + 1], spads[i], 128,
                          lambda t, i=i: resw_t[:, i * 9 + t, :],
                          resb_t[:, 0, i:i + 1], resb_t[:, 1, i:i + 1],
                          resb_t[:, 2, i:i + 1], DIL[i], spads[i])

            # fusion 1x1 (1024->256) + per-poly max over V (+ fus bias)
            gmax = [sn.tile([128, PADQ], f32, tag=f"gmax{m}", name=f"gmax{m}")
                    for m in range(2)]
            gb = [sn.tile([128, PADQ], bf16, tag=f"gb{m}", name=f"gb{m}")
                  for m in range(2)]
            for m in range(2):
                for qb in range(NQB):
                    ps = ppS.tile([128, 512], f32, tag="psS", name="psS")
                    for k in range(8):
                        sp = spads[k]
                        rhs = bass.AP(tensor=sp.tensor,
                                      offset=sp.offset + qb * 4 * PADV + 16,
                                      ap=[sp.ap[0], [PADV, 4], [1, 128]])
                        nc.tensor.matmul(ps, fusw_t[:, k, m * 128:(m + 1) * 128], rhs,
                                         start=(k == 0), stop=(k == 7))
                    nc.vector.tensor_reduce(gmax[m][:, qb * 4:(qb + 1) * 4],
                                            ps.rearrange("p (a b) -> p a b", a=4),
                                            axis=mybir.AxisListType.X, op=ALU.max)
                nc.vector.tensor_scalar(gb[m], gmax[m], fusb_t[:, m:m + 1], None,
                                        op0=ALU.add)

            # pred1: 1280 -> 256, relu
            h1 = [sn.tile([128, PADQ * 128], bf16, tag=f"h1_{m}", name=f"h1_{m}")
                  for m in range(2)]
            for m in range(2):
                for qb in range(NQB):
                    ps = ppS.tile([128, 512], f32, tag="psS", name="psS")
                    for k in range(10):
                        if k < 2:
                            rhs = _bcast(gb[k][:, qb * 4:(qb + 1) * 4], 128)
                        else:
                            sp = spads[k - 2]
                            rhs = bass.AP(tensor=sp.tensor,
                                          offset=sp.offset + qb * 4 * PADV + 16,
                                          ap=[sp.ap[0], [PADV, 4], [1, 128]])
                        nc.tensor.matmul(ps, pw1_t[:, k, m * 128:(m + 1) * 128], rhs,
                                         start=(k == 0), stop=(k == 9))
                    nc.scalar.activation(h1[m][:, qb * 512:(qb + 1) * 512], ps,
                                         AF.Relu, bias=pb1_t[:, m:m + 1])

            # pred2: 256 -> 64, relu
            h2 = sn.tile([64, PADQ * 128], bf16, tag="h2", name="h2")
            for qb in range(NQB):
                ps = ppT.tile([64, 512], f32, tag="psT", name="psT")
                for k in range(2):
                    nc.tensor.matmul(ps, pw2_t[:, k, :],
                                     h1[k][:, qb * 512:(qb + 1) * 512],
                                     start=(k == 0), stop=(k == 1))
                nc.scalar.activation(h2[:, qb * 512:(qb + 1) * 512], ps, AF.Relu,
                                     bias=pb2_t[:, 0:1])

            # pred3: 64 -> 2 per poly -> [128 v, PADQ, 2]
            ps3 = ppT.tile([128, PADQ * 2], f32, tag="psT3", name="psT3", bufs=1)
            for j in range(PADQ):
                nc.tensor.matmul(ps3[:, j * 2:(j + 1) * 2],
                                 h2[:, j * 128:(j + 1) * 128], pw3_t[:, :],
                                 start=True, stop=True)
            o_t = sn.tile([128, PADQ, 2], f32, tag="o_t", name="o_t")
            nc.vector.tensor_tensor(o_t, ps3.rearrange("p (a b) -> p a b", b=2),
                                    base_t, ALU.add)
            nc.sync.dma_start(out=d_out[:, :, :], in_=o_t)

    nc.compile()
    return nc


_NC_CACHE = {}


def _get_nc(P):
    if P not in _NC_CACHE:
        _NC_CACHE[P] = build_nc(P)
    return _NC_CACHE[P]


def _host_prep(inputs, P, counts, order, offs):
    """Build per-core in_maps."""
    cnn = np.asarray(inputs["cnn_feature"], np.float32)
    ipoly = np.asarray(inputs["i_it_poly"], np.float32)
    cpoly = np.asarray(inputs["c_it_poly"], np.float32)
    w1 = np.asarray(inputs["proj_w1"], np.float32)
    b2 = np.asarray(inputs["proj_b2"], np.float32)
    w2 = np.asarray(inputs["proj_w2"], np.float32)[:, :, 0, 0]  # [64, 256]
    NV = P * 128
    PADQ = -(-P // 4) * 4

    # ---- grid-sample host math (fp32, matches reference) ----
    ix = ipoly[..., 0] - np.float32(0.5)
    iy = ipoly[..., 1] - np.float32(0.5)
    x0 = np.floor(ix); y0 = np.floor(iy)
    wx = (ix - x0).astype(np.float32); wy = (iy - y0).astype(np.float32)
    x0i = x0.astype(np.int64); y0i = y0.astype(np.int64)
    corner_r = []; corner_w = []
    for dy, dx in ((0, 0), (0, 1), (1, 0), (1, 1)):
        xi = x0i + dx; yi = y0i + dy
        valid = (xi >= 0) & (xi < W) & (yi >= 0) & (yi < H)
        xc = np.clip(xi, 0, W - 1); yc = np.clip(yi, 0, H - 1)
        hw = yc * W + xc
        jt = hw // 128; p = hw % 128
        r = (jt // 8) * 1024 + p * 8 + (jt % 8)      # feat_dram row remap
        wgt = (wx if dx else (1 - wx)) * (wy if dy else (1 - wy))
        corner_r.append(r.astype(np.int64))
        corner_w.append((wgt * valid).astype(np.float32))
    s_v = np.sum(corner_w, axis=0)                    # [NP, V]

    # ---- shared packed weights ----
    w1p0 = np.zeros((128, 3, 2, 128), np.float32)
    w1p1 = np.zeros((70, 3, 2, 128), np.float32)
    for r0 in range(128):
        kh, ci = (0, r0) if r0 < 66 else (1, r0 - 66)
        for kw in range(3):
            for m in range(2):
                w1p0[r0, kw, m, :] = w1[m * 128:(m + 1) * 128, ci, kh, kw]
    for r1 in range(70):
        kh, ci = (1, 62 + r1) if r1 < 4 else (2, r1 - 4)
        for kw in range(3):
            for m in range(2):
                w1p1[r1, kw, m, :] = w1[m * 128:(m + 1) * 128, ci, kh, kw]
    w2t = np.transpose(w2, (1, 0)).reshape(2, 128, 64).transpose(1, 0, 2)

    headw = np.transpose(np.asarray(inputs["head_w"], np.float32), (1, 2, 0))
    headb = np.stack([np.asarray(inputs["head_b"], np.float32),
                      np.asarray(inputs["head_g"], np.float32),
                      np.asarray(inputs["head_bt"], np.float32)], axis=1)
    resw = np.transpose(np.asarray(inputs["res_w"], np.float32), (2, 0, 3, 1))
    resw = resw.reshape(128, 63, 128)
    resb = np.stack([np.asarray(inputs["res_b"], np.float32).T,
                     np.asarray(inputs["res_g"], np.float32).T,
                     np.asarray(inputs["res_bt"], np.float32).T], axis=1)
    fusw = np.transpose(np.asarray(inputs["fus_w"], np.float32).reshape(256, 8, 128),
                        (2, 1, 0))
    pw1 = np.transpose(np.asarray(inputs["pw1"], np.float32).reshape(256, 10, 128),
                       (2, 1, 0))
    pb1 = np.asarray(inputs["pb1"], np.float32).reshape(2, 128).T
    pw2 = np.transpose(np.asarray(inputs["pw2"], np.float32).reshape(64, 2, 128),
                       (2, 1, 0))
    pb2 = np.asarray(inputs["pb2"], np.float32).reshape(64, 1)
    pw3 = np.asarray(inputs["pw3"], np.float32).T
    pb3 = np.asarray(inputs["pb3"], np.float32)
    pb0 = np.asarray(inputs["proj_b1"], np.float32).reshape(2, 128).T
    fusb = np.asarray(inputs["fus_b"], np.float32).reshape(2, 128).T

    shared = {
        "w1p0": w1p0.astype(BF), "w1p1": w1p1.astype(BF), "w2t": w2t.astype(BF),
        "pb0": pb0, "fusb": fusb,
        "headw": headw.astype(BF), "headb": headb,
        "resw": resw.astype(BF), "resb": resb,
        "fusw": fusw.astype(BF), "pw1": pw1.astype(BF), "pb1": pb1,
        "pw2": pw2.astype(BF), "pb2": pb2, "pw3": pw3.astype(BF),
    }

    def pack16(idx_flat, cols):
        tab = np.zeros((16, cols), np.int16)
        n = len(idx_flat)
        tab[np.arange(n) % 16, np.arange(n) // 16] = idx_flat.astype(np.int16)
        return np.tile(tab, (8, 1))

    ind = np.asarray(inputs["ind"]).astype(np.int64)
    in_maps = []
    for c in range(N_CORES):
        img = cnn[c]
        img_pad = np.zeros((C_IN, PADW, PADW), np.float32)
        img_pad[:, 1:129, 1:129] = img
        flat = img_pad.reshape(C_IN, PIMG)
        stack0 = np.zeros((128, PIMG), np.float32)
        stack1 = np.zeros((70, PIMG), np.float32)
        stack0[0:66] = flat
        stack0[66:128, :PIMG - 130] = flat[0:62, 130:]
        stack1[0:4, :PIMG - 130] = flat[62:66, 130:]
        stack1[4:70, :PIMG - 260] = flat[0:66, 260:]

        own = order[offs[c]:offs[c + 1]]
        nown = len(own)
        idxc = np.zeros((4, NV), np.int64)
        wcomp = np.zeros((128, 4, P), np.float32)
        for cc in range(4):
            idxc[cc, :nown * 128] = corner_r[cc][own].reshape(-1)
            wcomp[:, cc, :nown] = corner_w[cc][own].T
        b2s = np.zeros((128, P, 64), np.float32)
        b2s[:, :nown, :] = s_v[own].T[:, :, None] * b2[None, None, :]
        coords = np.zeros((128, P, 2), np.float32)
        coords[:, :nown, :] = (cpoly[own] * RO).transpose(1, 0, 2)

        iidx = np.zeros(PADQ * PADV, np.int64)
        kk = np.arange(PADV)
        for q in range(nown):
            iidx[q * PADV:(q + 1) * PADV] = q * 128 + (kk + 112) % 128
        base = np.zeros((128, PADQ, 2), np.float32)
        if nown:
            base[:, :nown, :] = (ipoly[own] * RO + pb3[None, None, :]) \
                .transpose(1, 0, 2).astype(np.float32)

        m = {
            "stack0": stack0.astype(BF), "stack1": stack1.astype(BF),
            "idxc": np.stack([pack16(idxc[cc], NV // 16) for cc in range(4)], axis=1),
            "wcomp": wcomp, "b2s": b2s, "coords": coords.astype(BF),
            "iidx": pack16(iidx, PADQ * PADV // 16),
            "base": base,
        }
        m.update(shared)
        in_maps.append(m)
    return in_maps


def kernel(**inputs):
    ind = np.asarray(inputs["ind"]).astype(np.int64)
    counts = np.bincount(ind, minlength=N_CORES)
    P = int(counts.max())
    assert P <= 31, f"per-image poly count {P} exceeds int16 gather range"
    order = np.argsort(ind, kind="stable")
    offs = np.concatenate([[0], np.cumsum(counts)])

    nc = _get_nc(P)
    in_maps = _host_prep(inputs, P, counts, order, offs)
    res = None
    last_err = None
    for _attempt in range(3):
        try:
            res = run_bass_kernel_spmd(nc, in_maps, list(range(N_CORES)))
            break
        except Exception as e:  # rare transient device error; retry
            last_err = e
    if res is None:
        raise last_err

    out = np.zeros((NP, V, 2), np.float32)
    for c in range(N_CORES):
        oc = res.results[c]["out"]  # [128v, PADQ, 2]
        own = order[offs[c]:offs[c + 1]]
        for q, opoly in enumerate(own):
            out[opoly] = oc[:, q, :]
    return out



# revision 9
# speedup vs baseline: 2.3070x; 2.3070x over previous
"""Trainium2 Bass kernel for nn_Evolution_26697516712465 (deep-snake GNN).

Self-contained: takes FULL inputs, shards batch across 8 NeuronCores internally
(one image per core; each core runs the snake for the polys of its own image),
returns FULL output [128, 128, 2] fp32.

fp8e4 (e4m3) DoubleRow matmuls throughout (2 contraction rows/cycle), weights
pre-scaled by 64 into fp8 normal range, activations carried at power-of-2
scales; bilinear grid-sample folded into PE "diagonal" matmuls on gathered
corner row-pairs; eval-mode bn folded into weights/biases host-side.
"""
import numpy as np
import ml_dtypes
from contextlib import ExitStack

import concourse.bass as bass
import concourse.bacc as bacc
import concourse.mybir as mybir
import concourse.tile as tile
from concourse.library_config import mlp as mlp_lib
from concourse.bass_utils import run_bass_kernel_spmd

N_CORES = 8
B, C_IN, H, W = 8, 66, 128, 128
NP, V = 128, 128
RO = 4.0
DIL = (1, 1, 1, 2, 2, 4, 4)
NRES = 7
HW = H * W          # 16384
PADW = W + 2        # 130
PIMG = PADW * PADW  # 16900
PADV = 160          # 16 + 128 + 16 circular pad

f32 = mybir.dt.float32
f32r = mybir.dt.float32r
fp8 = mybir.dt.float8e4
i16 = mybir.dt.int16
AF = mybir.ActivationFunctionType
ALU = mybir.AluOpType
DR = mybir.MatmulPerfMode.DoubleRow

F8 = ml_dtypes.float8_e4m3

# activation/weight scales (powers of 2)
A_W = 64.0          # weight scale
S_R1 = 8.0          # conv1 relu out
S_FEAT = 32.0       # ipad feat rows (folded into diag weights)
S_Z = 32.0          # snake states
S_GB = 128.0        # fusion global feature
S_H1 = 128.0
S_H2 = 512.0

# conv1 stack row maps: blkA = 66ch kh0 + 33ch kh1; blkB = 33ch kh1 + 66ch kh2
ROWMAP_A = [(r, 0) if r < 66 else (r - 66, 1) for r in range(99)]
ROWMAP_B = [(r + 33, 1) if r < 33 else (r - 33, 2) for r in range(99)]


def _f8(x):
    return np.clip(np.asarray(x, np.float32), -240.0, 240.0).astype(F8)


def pack16(idx_flat, cols):
    tab = np.zeros((16, cols), np.int16)
    n = len(idx_flat)
    tab[np.arange(n) % 16, np.arange(n) // 16] = idx_flat.astype(np.int16)
    return np.tile(tab, (8, 1))


def build_nc(P, with_b2, zb):
    """Build the SPMD Bass program. P = max polys per image; zb = all relu
    biases are zero (allows relu on DVE/Pool engines)."""
    nc = bacc.Bacc("TRN2", target_bir_lowering=False, debug=False)
    PADQ = -(-P // 4) * 4
    NQB = PADQ // 4
    NV = PADQ * 128

    # ---------------- inputs ----------------
    d_stk = nc.declare_dram_parameter("stk", [99, 2, PIMG], fp8, isOutput=False)
    d_w1p = nc.declare_dram_parameter("w1p", [99, 3, 2, 2, 128], fp8, isOutput=False)
    d_pb0 = nc.declare_dram_parameter("pb0", [128, 2], f32, isOutput=False)
    d_w2t = nc.declare_dram_parameter("w2t", [128, 2, 128], fp8, isOutput=False)
    d_gixa = nc.declare_dram_parameter("gixa", [128, NV // 16], i16, isOutput=False)
    d_gixb = nc.declare_dram_parameter("gixb", [128, NV // 16], i16, isOutput=False)
    d_dgt = nc.declare_dram_parameter("dgt", [128, PADQ, 4, 128], fp8, isOutput=False)
    d_cpv = nc.declare_dram_parameter("cpv", [2, PADQ, 160], fp8, isOutput=False)
    d_headw = nc.declare_dram_parameter("headw", [66, 5, 2, 128], fp8, isOutput=False)
    d_lsb = nc.declare_dram_parameter("lsb", [128, 8, 2], f32, isOutput=False)
    d_resw = nc.declare_dram_parameter("resw", [128, 7, 5, 2, 128], fp8, isOutput=False)
    d_fusw = nc.declare_dram_parameter("fusw", [128, 4, 2, 2, 128], fp8, isOutput=False)
    d_fusc = nc.declare_dram_parameter("fusc", [128, 2], f32, isOutput=False)
    d_pw1 = nc.declare_dram_parameter("pw1", [128, 5, 2, 2, 128], fp8, isOutput=False)
    d_pb1 = nc.declare_dram_parameter("pb1", [128, 2], f32, isOutput=False)
    d_pw2 = nc.declare_dram_parameter("pw2", [128, 2, 64], fp8, isOutput=False)
    d_pb2 = nc.declare_dram_parameter("pb2", [64, 1], f32, isOutput=False)
    d_pw3 = nc.declare_dram_parameter("pw3", [64, 2], fp8, isOutput=False)
    d_base = nc.declare_dram_parameter("base", [128, PADQ, 2], f32, isOutput=False)
    if with_b2:
        d_b2r = nc.declare_dram_parameter("b2r", [1, 64], fp8, isOutput=False)
        d_svr = nc.declare_dram_parameter("svr", [1, PADQ, 128], fp8, isOutput=False)
    d_out = nc.declare_dram_parameter("out", [128, PADQ, 2], f32, isOutput=True)

    feat_dram = nc.dram_tensor("feat_dram", [HW, 64], f32)

    with tile.TileContext(nc, num_cores=N_CORES) as tc, ExitStack() as top:
        wpool = top.enter_context(tc.tile_pool(name="weights", bufs=1))
        # small weights first on the DMA queue
        w1p_t = wpool.tile([99, 3, 2, 2, 128], fp8)
        nc.sync.dma_start(out=w1p_t, in_=d_w1p[:, :, :, :, :])
        pb0_t = wpool.tile([128, 2], f32)
        nc.sync.dma_start(out=pb0_t, in_=d_pb0[:, :])
        w2t_t = wpool.tile([128, 2, 128], fp8)
        nc.sync.dma_start(out=w2t_t, in_=d_w2t[:, :, :])
        gixa_t = wpool.tile([128, NV // 16], i16)
        nc.sync.dma_start(out=gixa_t, in_=d_gixa[:, :])
        gixb_t = wpool.tile([128, NV // 16], i16)
        nc.sync.dma_start(out=gixb_t, in_=d_gixb[:, :])
        headw_t = wpool.tile([66, 5, 2, 128], fp8)
        nc.sync.dma_start(out=headw_t, in_=d_headw[:, :, :, :])
        lsb_t = wpool.tile([128, 8, 2], f32)
        nc.sync.dma_start(out=lsb_t, in_=d_lsb[:, :, :])
        base_t = wpool.tile([128, PADQ, 2], f32)
        nc.sync.dma_start(out=base_t, in_=d_base[:, :, :])
        fusc_t = wpool.tile([128, 2], f32)
        nc.sync.dma_start(out=fusc_t, in_=d_fusc[:, :])
        pb1_t = wpool.tile([128, 2], f32)
        nc.sync.dma_start(out=pb1_t, in_=d_pb1[:, :])
        pb2_t = wpool.tile([64, 1], f32)
        nc.sync.dma_start(out=pb2_t, in_=d_pb2[:, :])
        pw3_t = wpool.tile([64, 2], fp8)
        nc.sync.dma_start(out=pw3_t, in_=d_pw3[:, :])
        if with_b2:
            b2r_t = wpool.tile([1, 64], fp8)
            nc.sync.dma_start(out=b2r_t, in_=d_b2r[:, :])
            svr_t = wpool.tile([1, PADQ, 128], fp8)
            nc.sync.dma_start(out=svr_t, in_=d_svr[:, :, :])

        nc.gpsimd.load_library(mlp_lib)

        # big/late weights
        stk_t = wpool.tile([99, 2, PIMG], fp8)
        CHK = 16 * PADW
        for bb in range(8):
            nc.sync.dma_start(out=stk_t[:, :, bb * CHK:(bb + 1) * CHK],
                              in_=d_stk[:, :, bb * CHK:(bb + 1) * CHK])
        dgt_t = wpool.tile([128, PADQ, 4, 128], fp8)
        nc.sync.dma_start(out=dgt_t, in_=d_dgt[:, :, :, :])
        resw_t = wpool.tile([128, 7, 5, 2, 128], fp8)
        nc.sync.dma_start(out=resw_t, in_=d_resw[:, :, :, :, :])
        fusw_t = wpool.tile([128, 4, 2, 2, 128], fp8)
        nc.sync.dma_start(out=fusw_t, in_=d_fusw[:, :, :, :, :])
        pw1_t = wpool.tile([128, 5, 2, 2, 128], fp8)
        nc.sync.dma_start(out=pw1_t, in_=d_pw1[:, :, :, :, :])
        pw2_t = wpool.tile([128, 2, 64], fp8)
        nc.sync.dma_start(out=pw2_t, in_=d_pw2[:, :, :])

        # --- engine helpers ---
        rr_state = [0]

        def rr_relu(out_ap, in_ap, scale, bias_ap, force=None):
            """out = relu(scale*x + bias). scale may be const or AP.
            If zb (bias==0), can run on act or dve; else activation only.
            (GPSIMD cannot access PSUM, so pool never does these.)"""
            if not zb or force == 'act':
                nc.scalar.activation(out_ap, in_ap, AF.Relu,
                                     bias=(bias_ap if (bias_ap is not None and
                                                       not zb) else 0.0),
                                     scale=scale)
                return
            e = force if force is not None else ('act', 'dve')[rr_state[0] % 2]
            if force is None:
                rr_state[0] += 1
            if e == 'act':
                nc.scalar.activation(out_ap, in_ap, AF.Relu, bias=0.0,
                                     scale=scale)
            else:
                nc.vector.tensor_scalar(out_ap, in_ap, scale, 0.0,
                                        op0=ALU.mult, op1=ALU.max)

        def rr_copy(out_ap, in_ap, scale=None, force=None, pool_ok=False):
            engs = ('act', 'dve', 'pool') if pool_ok else ('act', 'dve')
            e = force if force is not None else engs[rr_state[0] % len(engs)]
            if force is None:
                rr_state[0] += 1
            if scale is None:
                if e == 'act':
                    nc.scalar.activation(out_ap, in_ap, AF.Copy, bias=0.0)
                elif e == 'dve':
                    nc.vector.tensor_copy(out_ap, in_ap)
                else:
                    nc.gpsimd.tensor_copy(out_ap, in_ap)
            else:
                if e == 'act':
                    nc.scalar.activation(out_ap, in_ap, AF.Copy, bias=0.0,
                                         scale=scale)
                elif e == 'dve':
                    nc.vector.tensor_scalar(out_ap, in_ap, scale, None,
                                            op0=ALU.mult)
                else:
                    nc.gpsimd.tensor_scalar(out_ap, in_ap, scale, None,
                                            op0=ALU.mult)

        # ------------ conv1 (3x3, 66->256) + conv2 (1x1, 256->64) ------------
        with tc.tile_pool(name="conv", bufs=1) as cpool, \
             tc.tile_pool(name="psumA", bufs=2, space="PSUM") as ppA, \
             tc.tile_pool(name="psumB", bufs=2, space="PSUM") as ppB, \
             tc.tile_pool(name="stage", bufs=3) as spool:
            r1 = cpool.tile([128, 2, HW], fp8)
            ps2 = {}

            def emit_conv2(g):
                h = g % 2
                if h == 0:
                    ps2[0] = ppB.tile([128, 2, 512], f32, tag="c2", name="c2")
                rhs2 = bass.AP(tensor=r1.tensor, offset=r1.offset + g * 512,
                               ap=[r1.ap[0], [HW, 2], [1, 512]])
                nc.tensor.matmul(ps2[0][:, h, :], w2t_t[:, :, :],
                                 rhs2, start=True, stop=True, perf_mode=DR)
                if h == 1:
                    stg = spool.tile([64, 2, 512], f32, tag="stage", name="stg")
                    rr_copy(stg, ps2[0][0:64, :, :], 1.0 / (A_W * S_R1))
                    dst = bass.AP(tensor=feat_dram,
                                  offset=((g - 1) * 512) * 64,
                                  ap=[[1, 64], [64, 1024]])
                    nc.sync.dma_start(out=dst, in_=stg)

            for g in range(32):              # y-groups of 4 rows
                for m in range(2):
                    pg = ppA.tile([128, 4, 128], f32, tag=f"c1_{m}",
                                  name=f"c1_{m}")
                    for yy in range(4):
                        y = 4 * g + yy
                        ktaps = ((y * PADW, 1), (y * PADW + 2, PIMG - 2),
                                 (PIMG + y * PADW + 1, 1))
                        for p, (off, stride) in enumerate(ktaps):
                            rhs = bass.AP(tensor=stk_t.tensor,
                                          offset=stk_t.offset + off,
                                          ap=[stk_t.ap[0], [stride, 2],
                                              [1, 128]])
                            nc.tensor.matmul(pg[:, yy, :], w1p_t[:, p, :, m, :],
                                             rhs, start=(p == 0), stop=(p == 2),
                                             perf_mode=DR)
                    rr_relu(r1[:, m, g * 512:(g + 1) * 512],
                            pg.rearrange("p a b -> p (a b)"), S_R1 / A_W,
                            pb0_t[:, m:m + 1])
                if g >= 2:
                    emit_conv2(g - 2)        # skewed to avoid PE queue stall
            emit_conv2(30)
            emit_conv2(31)

        # ------------ gather + combine-transpose into ipad ------------
        with tc.tile_pool(name="snake", bufs=1) as sn:
            ipad = sn.tile([128, PADQ, PADV], fp8, tag="ipad", name="ipad")
            zall = sn.tile([128, 8, PADQ, PADV], fp8, tag="zall", name="zall")
            rsc = sn.tile([128, PADQ, 128], fp8, tag="rsc", name="rsc")

            with tc.tile_pool(name="gat", bufs=1) as gp, \
                 tc.tile_pool(name="psumG", bufs=6, space="PSUM") as ppG:
                gta = gp.tile([128, PADQ, 128], f32, tag="gta", name="gta")
                gtb = gp.tile([128, PADQ, 128], f32, tag="gtb", name="gtb")
                gsrc = bass.AP(tensor=feat_dram, offset=0,
                               ap=[[64, HW - 1], [1, 128]])
                nc.gpsimd.dma_gather(gta, gsrc, gixa_t[:, :], NV, NV, 128,
                                     elem_step=64, single_packet=False)
                nc.gpsimd.dma_gather(gtb, gsrc, gixb_t[:, :], NV, NV, 128,
                                     elem_step=64, single_packet=False)
                nc.sync.dma_start(out=ipad[64:66, :, :], in_=d_cpv[:, :, :])

                # convert gathered corners f32 -> fp8 (x S_FEAT)
                g8a = gp.tile([128, PADQ, 128], fp8, tag="g8a", name="g8a")
                g8b = gp.tile([128, PADQ, 128], fp8, tag="g8b", name="g8b")
                for src, dst in ((gta, g8a), (gtb, g8b)):
                    for qb in range(NQB):
                        qsl = slice(4 * qb, 4 * qb + 4)
                        rr_copy(dst[:, qsl, :], src[:, qsl, :],
                                scale=S_FEAT, pool_ok=True)

                for qb in range(NQB):
                    pg = ppG.tile([64, 4, 128], f32, tag="dg", name="dg")
                    for qq in range(4):
                        q = 4 * qb + qq
                        last = 2 if with_b2 else 1
                        for gi, gt in enumerate((g8a, g8b)):
                            lhsT = bass.AP(
                                tensor=gt.tensor,
                                offset=gt.offset + q * 128,
                                ap=[gt.ap[0], [64, 2], [1, 64]])
                            nc.tensor.matmul(pg[:, qq, :], lhsT,
                                             dgt_t[:, q, 2 * gi:2 * gi + 2, :],
                                             start=(gi == 0), stop=(gi == last),
                                             perf_mode=DR)
                        if with_b2:
                            nc.tensor.matmul(pg[:, qq, :], b2r_t[:, :],
                                             svr_t[:, q, :], start=False,
                                             stop=True)
                    rr_copy(ipad[0:64, 4 * qb:4 * qb + 4, 16:144], pg)
                nc.vector.tensor_copy(ipad[0:66, :, 0:16],
                                      ipad[0:66, :, 128:144])
                nc.gpsimd.tensor_copy(ipad[0:66, :, 144:160],
                                      ipad[0:66, :, 16:32])

            # ------------ snake ------------
            with tc.tile_pool(name="psumS", bufs=4, space="PSUM") as ppS:

                def conv_layer(zo, rhs_base_fn, rhs_tensor, rhs_ap0, lhsT_fn,
                               dil, src_zi):
                    """One circular conv layer; per-qb relu (+residual add)."""
                    for qb in range(NQB):
                        ps = ppS.tile([128, 4, 128], f32, tag="psS", name="psS")
                        for qq in range(4):
                            q = 4 * qb + qq
                            for p in range(5):
                                if p < 4:
                                    off = rhs_base_fn(q) + 16 + (2 * p - 4) * dil
                                    stride = dil
                                else:
                                    off = rhs_base_fn(q) + 16 + 4 * dil
                                    stride = 0
                                rhs = bass.AP(tensor=rhs_tensor, offset=off,
                                              ap=[rhs_ap0, [stride, 2],
                                                  [1, 128]])
                                nc.tensor.matmul(ps[:, qq, :], lhsT_fn(p), rhs,
                                                 start=(p == 0), stop=(p == 4),
                                                 perf_mode=DR)
                        qsl = slice(4 * qb, 4 * qb + 4)
                        scale_ap = lsb_t[:, zo, 0:1]
                        bias_ap = lsb_t[:, zo, 1:2]
                        if src_zi is None:
                            # head: write z0 directly
                            rr_relu(zall[:, 0, qsl, 16:144], ps, scale_ap,
                                    bias_ap, force=('act' if qb != 4 else 'dve'))
                        else:
                            rr_relu(rsc[:, qsl, :], ps, scale_ap, bias_ap,
                                    force=('act' if qb != 4 else 'dve'))
                            nc.vector.tensor_tensor(
                                zall[:, zo, qsl, 16:144],
                                zall[:, src_zi, qsl, 16:144],
                                rsc[:, qsl, :], ALU.add)
                        # wraps for this layer's output, per qb
                        weng = nc.vector if qb % 2 == 0 else nc.gpsimd
                        weng.tensor_copy(zall[:, zo, qsl, 0:16],
                                         zall[:, zo, qsl, 128:144])
                        weng.tensor_copy(zall[:, zo, qsl, 144:160],
                                         zall[:, zo, qsl, 16:32])

                ip66 = ipad[0:66, :, :]
                conv_layer(0, lambda q: ip66.offset + q * PADV, ip66.tensor,
                           ip66.ap[0], lambda p: headw_t[:, p, :, :], 1, None)
                for i in range(NRES):
                    zi_off = zall.offset + i * PADQ * PADV
                    conv_layer(i + 1,
                               lambda q, zi_off=zi_off: zi_off + q * PADV,
                               zall.tensor, zall.ap[0],
                               lambda p, i=i: resw_t[:, i, p, :, :],
                               DIL[i], i)

                # fusion 1x1 (1024->256) + per-poly max over V
                gmax = sn.tile([128, 2, PADQ], f32, tag="gmax", name="gmax")
                gb = sn.tile([128, 2, PADQ], fp8, tag="gb", name="gb")
                for m in range(2):
                    for qb in range(NQB):
                        ps = ppS.tile([128, 4, 128], f32, tag="psS", name="psS")
                        for qq in range(4):
                            q = 4 * qb + qq
                            for k in range(4):
                                off = zall.offset + (2 * k * PADQ + q) * PADV + 16
                                rhs = bass.AP(tensor=zall.tensor, offset=off,
                                              ap=[zall.ap[0],
                                                  [PADQ * PADV, 2], [1, 128]])
                                nc.tensor.matmul(ps[:, qq, :],
                                                 fusw_t[:, k, :, m, :], rhs,
                                                 start=(k == 0), stop=(k == 3),
                                                 perf_mode=DR)
                        nc.vector.tensor_reduce(gmax[:, m, 4 * qb:4 * qb + 4],
                                                ps, axis=mybir.AxisListType.X,
                                                op=ALU.max)
                    nc.vector.tensor_scalar(gb[:, m, :], gmax[:, m, :],
                                            S_GB / (A_W * S_Z),
                                            fusc_t[:, m:m + 1],
                                            op0=ALU.mult, op1=ALU.add)

                # pred1: 1280 -> 256 relu
                h1 = sn.tile([128, 2, PADQ, 128], fp8, tag="h1", name="h1")
                for m in range(2):
                    for qb in range(NQB):
                        ps = ppS.tile([128, 4, 128], f32, tag="psS", name="psS")
                        for qq in range(4):
                            q = 4 * qb + qq
                            rhs0 = bass.AP(tensor=gb.tensor,
                                           offset=gb.offset + q,
                                           ap=[gb.ap[0], [PADQ, 2], [0, 128]])
                            nc.tensor.matmul(ps[:, qq, :], pw1_t[:, 0, :, m, :],
                                             rhs0, start=True, stop=False,
                                             perf_mode=DR)
                            for k in range(4):
                                off = zall.offset + (2 * k * PADQ + q) * PADV + 16
                                rhs = bass.AP(tensor=zall.tensor, offset=off,
                                              ap=[zall.ap[0],
                                                  [PADQ * PADV, 2], [1, 128]])
                                nc.tensor.matmul(ps[:, qq, :],
                                                 pw1_t[:, k + 1, :, m, :], rhs,
                                                 start=False, stop=(k == 3),
                                                 perf_mode=DR)
                        rr_relu(h1[:, m, 4 * qb:4 * qb + 4, :], ps,
                                S_H1 / (A_W * S_Z), pb1_t[:, m:m + 1])

            # pred2 + pred3
            with tc.tile_pool(name="psumT", bufs=3, space="PSUM") as ppT:
                h2 = sn.tile([64, PADQ, 128], fp8, tag="h2", name="h2")
                for qb in range(NQB):
                    ps = ppT.tile([64, 4, 128], f32, tag="psT", name="psT")
                    for qq in range(4):
                        q = 4 * qb + qq
                        rhs = bass.AP(tensor=h1.tensor,
                                      offset=h1.offset + q * 128,
                                      ap=[h1.ap[0], [PADQ * 128, 2], [1, 128]])
                        nc.tensor.matmul(ps[:, qq, :], pw2_t[:, :, :], rhs,
                                         start=True, stop=True, perf_mode=DR)
                    rr_relu(h2[:, 4 * qb:4 * qb + 4, :], ps,
                            S_H2 / (A_W * S_H1), pb2_t[:, 0:1])

                ps3 = ppT.tile([128, PADQ, 2], f32, tag="psT3", name="psT3",
                               bufs=1)
                for q in range(PADQ):
                    nc.tensor.matmul(ps3[:, q, :], h2[:, q, :], pw3_t[:, :],
                                     start=True, stop=True)
                o_f = sn.tile([128, PADQ, 2], f32, tag="o_f", name="o_f")
                nc.vector.tensor_scalar(o_f, ps3, 1.0 / (A_W * S_H2), None,
                                        op0=ALU.mult)
                o_t = sn.tile([128, PADQ, 2], f32, tag="o_t", name="o_t")
                nc.vector.tensor_tensor(o_t, o_f, base_t, ALU.add)
                nc.sync.dma_start(out=d_out[:, :, :], in_=o_t)

    nc.compile()
    return nc


_NC_CACHE = {}


def _get_nc_key(P, with_b2, zb):
    key = (P, with_b2, zb)
    if key not in _NC_CACHE:
        _NC_CACHE[key] = build_nc(P, with_b2, zb)
    return _NC_CACHE[key]


def _get_nc(P):
    """test.py compatibility: default flags for the standard input set."""
    return _get_nc_key(P, False, True)


def _host_prep(inputs, P, counts, order, offs):
    """Build per-core in_maps. Returns (in_maps, with_b2, zb)."""
    PADQ = -(-P // 4) * 4
    NV = PADQ * 128
    cnn = np.asarray(inputs["cnn_feature"], np.float32)
    ipoly = np.asarray(inputs["i_it_poly"], np.float32)
    cpoly = np.asarray(inputs["c_it_poly"], np.float32)
    w1 = np.asarray(inputs["proj_w1"], np.float32)
    pb0 = np.asarray(inputs["proj_b1"], np.float32)
    w2 = np.asarray(inputs["proj_w2"], np.float32)[:, :, 0, 0]  # [64, 256]
    b2 = np.asarray(inputs["proj_b2"], np.float32)
    head_w = np.asarray(inputs["head_w"], np.float32)   # [128, 66, 9]
    head_b = np.asarray(inputs["head_b"], np.float32)
    head_g = np.asarray(inputs["head_g"], np.float32)
    head_bt = np.asarray(inputs["head_bt"], np.float32)
    res_w = np.asarray(inputs["res_w"], np.float32)     # [7, 128, 128, 9]
    res_b = np.asarray(inputs["res_b"], np.float32)
    res_g = np.asarray(inputs["res_g"], np.float32)
    res_bt = np.asarray(inputs["res_bt"], np.float32)
    fus_w = np.asarray(inputs["fus_w"], np.float32)     # [256, 1024]
    fus_b = np.asarray(inputs["fus_b"], np.float32)
    pw1 = np.asarray(inputs["pw1"], np.float32)         # [256, 1280]
    pb1 = np.asarray(inputs["pb1"], np.float32)
    pw2 = np.asarray(inputs["pw2"], np.float32)         # [64, 256]
    pb2 = np.asarray(inputs["pb2"], np.float32)
    pw3 = np.asarray(inputs["pw3"], np.float32)         # [2, 64]
    pb3 = np.asarray(inputs["pb3"], np.float32)

    assert (head_g > 0).all() and (res_g > 0).all(), "bn fold requires g>0"
    with_b2 = bool(np.any(b2 != 0))

    # w1p [99, 3pairs, 2kt, 2m, 128]
    w1p = np.zeros((99, 3, 2, 2, 128), np.float32)
    pair_src = [((0, 0), (0, 1)), ((0, 2), (1, 0)), ((1, 1), (1, 2))]
    for p, pr in enumerate(pair_src):
        for kt, (blk, kw) in enumerate(pr):
            rm = ROWMAP_A if blk == 0 else ROWMAP_B
            for r in range(99):
                ch, kh = rm[r]
                for m in range(2):
                    w1p[r, p, kt, m, :] = A_W * w1[m * 128:(m + 1) * 128,
                                                   ch, kh, kw]
    pb0s = (S_R1 * pb0).reshape(2, 128).T.copy()

    w2t = np.zeros((128, 2, 128), np.float32)
    for kt in range(2):
        w2t[:, kt, 0:64] = A_W * w2[:, kt * 128:(kt + 1) * 128].T
        w2t[:, kt, 64:128] = w2t[:, kt, 0:64]

    # ---- grid-sample host math ----
    ix = ipoly[..., 0] - np.float32(0.5)
    iy = ipoly[..., 1] - np.float32(0.5)
    x0 = np.floor(ix); y0 = np.floor(iy)
    wx = (ix - x0).astype(np.float32); wy = (iy - y0).astype(np.float32)
    x0i = x0.astype(np.int64); y0i = y0.astype(np.int64)

    swap_x = x0i < 0
    vx0 = (x0i >= 0) & (x0i < W)
    vx1 = (x0i + 1 >= 0) & (x0i + 1 < W)

    def slot_weights(yi):
        vy = (yi >= 0) & (yi < H)
        w_s0 = (1 - wx) * vx0 * vy
        w_s1 = wx * vx1 * vy
        w_s0 = np.where(swap_x, wx * vx1 * vy, w_s0)
        w_s1 = np.where(swap_x, 0.0, w_s1)
        return w_s0.astype(np.float32), w_s1.astype(np.float32)

    x0c = np.clip(x0i, 0, W - 2)
    y0c = np.clip(y0i, 0, H - 1)
    y1c = np.clip(y0i + 1, 0, H - 1)
    idxA = (y0c * W + x0c).astype(np.int64)          # [NP, V]
    idxB = (y1c * W + x0c).astype(np.int64)
    wA0, wA1 = slot_weights(y0i)
    wB0, wB1 = slot_weights(y0i + 1)
    wA0 *= (1 - wy); wA1 *= (1 - wy)
    wB0 *= wy; wB1 *= wy
    s_v = wA0 + wA1 + wB0 + wB1

    # ---- snake weights (bn + scale folds) ----
    headw = np.zeros((66, 5, 2, 128), np.float32)
    hw9 = head_w.transpose(1, 2, 0)                  # [66, 9, 128]
    for t in range(9):
        p, kt = t // 2, t % 2
        headw[0:64, p, kt, :] = A_W * hw9[0:64, t, :]
        headw[64:66, p, kt, :] = A_W * S_FEAT * hw9[64:66, t, :]
    lsb = np.zeros((128, 8, 2), np.float32)
    lsb[:, 0, 0] = head_g / A_W
    lsb[:, 0, 1] = S_Z * head_g * head_b
    C = np.zeros((8, 128), np.float32)               # C_i = sum_{j<=i} bt_j
    C[0] = head_bt
    for i in range(NRES):
        C[i + 1] = C[i] + res_bt[i]

    resw = np.zeros((128, 7, 5, 2, 128), np.float32)
    for i in range(NRES):
        rw = res_w[i].transpose(1, 2, 0)             # [128 in, 9, 128 out]
        for t in range(9):
            p, kt = t // 2, t % 2
            resw[:, i, p, kt, :] = A_W * rw[:, t, :]
        bprime = res_b[i] + res_w[i].sum(axis=2) @ C[i]
        lsb[:, i + 1, 0] = res_g[i] / A_W
        lsb[:, i + 1, 1] = S_Z * res_g[i] * bprime

    fw8 = fus_w.reshape(256, 8, 128)
    fusw = np.zeros((128, 4, 2, 2, 128), np.float32)
    for k in range(4):
        for kt in range(2):
            for m in range(2):
                fusw[:, k, kt, m, :] = A_W * fw8[m * 128:(m + 1) * 128,
                                                 2 * k + kt].T
    fusconst = fus_b + np.einsum('ojc,jc->o', fw8, C)
    fusc = (S_GB * fusconst).reshape(2, 128).T.copy()

    pw1r = pw1.reshape(256, 10, 128)
    pw1p = np.zeros((128, 5, 2, 2, 128), np.float32)
    for m in range(2):
        for kt in range(2):
            pw1p[:, 0, kt, m, :] = (A_W * S_Z / S_GB) * \
                pw1r[m * 128:(m + 1) * 128, kt].T
        for k in range(4):
            for kt in range(2):
                pw1p[:, k + 1, kt, m, :] = A_W * \
                    pw1r[m * 128:(m + 1) * 128, 2 + 2 * k + kt].T
    pb1prime = pb1 + np.einsum('ojc,jc->o', pw1r[:, 2:], C)
    pb1s = (S_H1 * pb1prime).reshape(2, 128).T.copy()

    pw2t = np.zeros((128, 2, 64), np.float32)
    for kt in range(2):
        pw2t[:, kt, :] = A_W * pw2[:, kt * 128:(kt + 1) * 128].T
    pb2s = (S_H2 * pb2).reshape(64, 1)
    pw3t = A_W * pw3.T                                # [64, 2]

    zb = (not np.any(pb0)) and (not np.any(lsb[:, :, 1])) \
        and (not np.any(pb1s)) and (not np.any(pb2s))

    shared = {
        "w1p": _f8(w1p), "pb0": pb0s, "w2t": _f8(w2t),
        "headw": _f8(headw), "lsb": lsb,
        "resw": _f8(resw), "fusw": _f8(fusw), "fusc": fusc,
        "pw1": _f8(pw1p), "pb1": pb1s, "pw2": _f8(pw2t), "pb2": pb2s,
        "pw3": _f8(pw3t),
    }
    if with_b2:
        shared["b2r"] = _f8(S_FEAT * b2.reshape(1, 64))

    in_maps = []
    for c in range(N_CORES):
        img = cnn[c]
        img_pad = np.zeros((C_IN, PADW, PADW), np.float32)
        img_pad[:, 1:129, 1:129] = img
        flatf = _f8(img_pad.reshape(C_IN, PIMG)).astype(np.float32)
        stk = np.zeros((99, 2, PIMG), np.float32)
        for r in range(99):
            ch, kh = ROWMAP_A[r]
            ln = PIMG - kh * PADW
            stk[r, 0, :ln] = flatf[ch, kh * PADW:]
            ch, kh = ROWMAP_B[r]
            ln = PIMG - kh * PADW
            stk[r, 1, :ln] = flatf[ch, kh * PADW:]

        own = order[offs[c]:offs[c + 1]]
        nown = len(own)
        gixa = np.zeros(NV, np.int64)
        gixb = np.zeros(NV, np.int64)
        dgt = np.zeros((128, PADQ, 4, 128), np.float32)
        ar = np.arange(128)
        for qi, poly in enumerate(own):
            gixa[qi * 128:(qi + 1) * 128] = idxA[poly]
            gixb[qi * 128:(qi + 1) * 128] = idxB[poly]
            dgt[ar, qi, 0, ar] = wA0[poly]
            dgt[ar, qi, 1, ar] = wA1[poly]
            dgt[ar, qi, 2, ar] = wB0[poly]
            dgt[ar, qi, 3, ar] = wB1[poly]

        cpv = np.zeros((2, PADQ, 160), np.float32)
        if nown:
            cc = (cpoly[own] * RO).transpose(2, 0, 1)     # [2, nown, 128]
            cpv[:, :nown, 16:144] = cc
            cpv[:, :nown, 0:16] = cc[:, :, 112:128]
            cpv[:, :nown, 144:160] = cc[:, :, 0:16]

        base = np.zeros((128, PADQ, 2), np.float32)
        if nown:
            base[:, :nown, :] = (ipoly[own] * RO + pb3[None, None, :]) \
                .transpose(1, 0, 2)

        m = {
            "stk": _f8(stk),
            "gixa": pack16(gixa, NV // 16), "gixb": pack16(gixb, NV // 16),
            "dgt": _f8(dgt), "cpv": _f8(cpv), "base": base,
        }
        if with_b2:
            svr = np.zeros((1, PADQ, 128), np.float32)
            svr[0, :nown, :] = s_v[own]
            m["svr"] = _f8(svr)
        m.update(shared)
        in_maps.append(m)
    return in_maps, with_b2, zb


def kernel(**inputs):
    ind = np.asarray(inputs["ind"]).astype(np.int64)
    counts = np.bincount(ind, minlength=N_CORES)
    P = int(counts.max())
    order = np.argsort(ind, kind="stable")
    offs = np.concatenate([[0], np.cumsum(counts)])

    in_maps, with_b2, zb = _host_prep(inputs, P, counts, order, offs)
    nc = _get_nc_key(P, with_b2, zb)
    res = None
    last_err = None
    for _attempt in range(3):
        try:
            res = run_bass_kernel_spmd(nc, in_maps, list(range(N_CORES)))
            break
        except Exception as e:  # rare transient device error; retry
            last_err = e
    if res is None:
        raise last_err

    out = np.zeros((NP, V, 2), np.float32)
    for c in range(N_CORES):
        oc = res.results[c]["out"]  # [128v, PADQ, 2]
        own = order[offs[c]:offs[c + 1]]
        for q, opoly in enumerate(own):
            out[opoly] = oc[:, q, :]
    return out
